# revision 10
# baseline (speedup 1.0000x reference)
"""KAN-GNN message passing on 8 TRN2 NeuronCores.

Strategy (data-parallel over nodes, per sharding hint):
 - Nodes ranked by in-degree, dealt round-robin to 8 cores (12544 local
   targets each, 98 windows of 128). The gathered tables are built as 4
   piece-wise AllGathers (windows 0-24, 25-49, 50-73, 74-97) so (a) each
   piece is <=25600 rows, addressable by the hardware dma_gather's int16
   indices, and (b) each collective overlaps with compute that produced
   or consumes the neighbouring pieces.
 - Phase 1: KAN layer 1 per local node (3 fused matmuls, bias+relu on
   DVE); AllGather piece p fires as soon as its windows are written.
 - Aggregation is edge-major: per core, in-edges sorted by (target
   window, source piece); each (w,c) run padded to a multiple of 128
   (shared across cores for SPMD) and fetched with hardware dma_gather
   (<=1024 rows per instruction, 4 SWDGE queues round-robin). A
   per-sub-batch 0/1 "binning" matrix B (fp8, streamed per window on the
   ACT engine's HWDGE) scatters each 128 gathered messages onto the
   window's 128 targets via one PE matmul accumulating in PSUM; the
   drain applies 1/deg on DVE.
 - KAN layer 2 per window (PE transpose, DVE powers, 3 matmuls); h2 kept
   f32 (256B rows) and AllGathered piece-wise during phase 2; second
   edge-major gather + binning pass; log_softmax with deferred Ln so the
   ACT engine loads each activation table once.
 - All indices/B matrices precomputed on host; per-core data as inputs.
"""
import numpy as np
import ml_dtypes

import concourse.bacc as bacc
import concourse.mybir as mybir
import concourse.tile as tile
import concourse.bass as bass
from concourse.bass_utils import run_bass_kernel_spmd

N_NODES = 100000
N_EDGES = 1600000
IN_F, HID_F, OUT_F = 128, 128, 64
K = 8               # cores
P = 128             # partitions
J = 12544           # local nodes per core (98*128), 12500 real + 44 pad
G = J // P          # 98 windows
JREAL = N_NODES // K
NCH = 4             # table pieces / gather chunks
PIECE_W = [25, 25, 24, 24]          # windows per piece
PIECE_W0 = [0, 25, 50, 74]          # first window of each piece
PIECE_ROWS = [w * P for w in PIECE_W]   # local rows per piece
RMAX = 8            # max sub-batches (of 128 edges) per gather piece

BF16 = mybir.dt.bfloat16
F32 = mybir.dt.float32
I16 = mybir.dt.int16
FP8 = mybir.dt.float8e4

_PIECE_OF_W = np.repeat(np.arange(NCH), PIECE_W)


def _host_prep(x, edge_index, w1, b1, c1, w2, b2, c2):
    src = np.asarray(edge_index[0], dtype=np.int64)
    tgt = np.asarray(edge_index[1], dtype=np.int64)
    x = np.asarray(x, dtype=np.float32)

    deg = np.bincount(tgt, minlength=N_NODES)
    order = np.argsort(-deg, kind="stable")
    rank_of = np.empty(N_NODES, dtype=np.int64)
    rank_of[order] = np.arange(N_NODES)
    core_of = rank_of % K
    j_of = rank_of // K

    # table position within its piece: rel = core*rows_p + (j - 128*w0_p)
    w_of = j_of // P
    piece_of = _PIECE_OF_W[w_of]
    rows_p = np.array(PIECE_ROWS)[piece_of]
    w0_p = np.array(PIECE_W0)[piece_of]
    rel_of = core_of * rows_p + (j_of - w0_p * P)

    # per-edge attributes
    ek = core_of[tgt]                    # owning core (by target)
    we = j_of[tgt] // P                  # target window
    ce = piece_of[src]                   # source chunk/piece
    rel = rel_of[src]                    # in-piece table row
    tcol = j_of[tgt] % P                 # target column within window

    key = (ek * G + we) * NCH + ce
    eorder = np.lexsort((rel, key))
    skey = key[eorder]
    counts = np.bincount(key, minlength=K * G * NCH).reshape(K, G, NCH)
    Lmax = counts.max(axis=0)                        # [G, NCH]
    Lpad = ((Lmax + P - 1) // P) * P                 # shared padded run lens

    run_off = np.zeros((G, NCH), dtype=np.int64)
    TOT = 0
    for w in range(G):
        for c in range(NCH):
            run_off[w, c] = TOT
            TOT += int(Lpad[w, c])
    SBTOT = TOT // P

    # place each core's sorted edges into the shared padded stream
    flat_counts = counts.reshape(-1)
    run_starts_e = np.concatenate([[0], np.cumsum(flat_counts)[:-1]])
    d_in_run = np.arange(len(skey)) - np.repeat(run_starts_e, flat_counts)
    kk = skey // (G * NCH)
    ww = (skey // NCH) % G
    cc = skey % NCH
    ppos = run_off[ww, cc] + d_in_run

    idx_rel = np.zeros((K, TOT), dtype=np.int16)     # pads -> row 0 (B=0)
    tcol_pad = np.full((K, TOT), -1, dtype=np.int64)
    idx_rel[kk, ppos] = rel[eorder].astype(np.int16)
    tcol_pad[kk, ppos] = tcol[eorder]

    # int16 index stream wrapped in 16 partitions, replicated x8 gpsimd cores
    blk = idx_rel.reshape(K, TOT // 16, 16).transpose(0, 2, 1)
    idx16 = np.ascontiguousarray(np.tile(blk, (1, 8, 1)))  # [K, 128, TOT//16]

    # binning matrices: Ball[k, p, s, t] = 1 if edge (s*128+p) targets t
    bm = np.zeros((K, TOT, P), dtype=ml_dtypes.float8_e4m3)
    kidx, eidx = np.nonzero(tcol_pad >= 0)
    bm[kidx, eidx, tcol_pad[kidx, eidx]] = 1.0
    Ball = np.ascontiguousarray(
        bm.reshape(K, SBTOT, P, P).transpose(0, 2, 1, 3))  # [K,128,SBTOT,128]

    # 1/deg per local target [K, P, G] (0 for pad targets)
    degs_kj = np.zeros((K, J), dtype=np.int64)
    degs_kj[core_of, j_of] = deg
    dr = 1.0 / np.maximum(degs_kj, 1).astype(np.float32)
    real = np.zeros((K, J), dtype=np.float32)
    real[:, :JREAL] = 1.0
    degrecip = np.ascontiguousarray(
        (dr * real).reshape(K, G, P).transpose(0, 2, 1))

    # xT shards, bf16 [K][IN_F, J]
    xT = np.zeros((K, IN_F, J), dtype=ml_dtypes.bfloat16)
    for k in range(K):
        nodes_k = order[np.arange(JREAL) * K + k]
        xT[k, :, :JREAL] = x[nodes_k].T.astype(ml_dtypes.bfloat16)

    # fused KAN weights
    A1 = (w1 + 0.1 * c1[:, :, 0]).astype(ml_dtypes.bfloat16)
    B1 = (0.1 * c1[:, :, 1]).astype(ml_dtypes.bfloat16)
    C1 = (0.1 * c1[:, :, 2]).astype(ml_dtypes.bfloat16)
    A2 = (w2 + 0.1 * c2[:, :, 0]).astype(ml_dtypes.bfloat16)
    B2 = (0.1 * c2[:, :, 1]).astype(ml_dtypes.bfloat16)
    C2 = (0.1 * c2[:, :, 2]).astype(ml_dtypes.bfloat16)
    b1b = np.tile(np.asarray(b1, np.float32)[None, :], (P, 1))
    b2b = np.tile(np.asarray(b2, np.float32)[None, :], (P, 1))
    ident = np.eye(P, dtype=np.float32)

    in_maps = []
    for k in range(K):
        in_maps.append({
            "xT": xT[k],
            "idx16": idx16[k],
            "Ball": Ball[k],
            "degrecip": degrecip[k],
            "A1": A1, "B1": B1, "C1": C1,
            "A2": A2, "B2": B2, "C2": C2,
            "b1b": b1b, "b2b": b2b, "ident": ident,
        })
    meta = {"Lpad": Lpad, "run_off": run_off, "TOT": TOT, "SBTOT": SBTOT,
            "order": order}
    return in_maps, meta


def build_program(meta):
    Lpad = meta["Lpad"]
    run_off = meta["run_off"]
    TOT = int(meta["TOT"])
    SBTOT = int(meta["SBTOT"])
    TOTC = TOT // 16

    nc = bacc.Bacc("TRN2", target_bir_lowering=False, debug=False, num_devices=K,
                   num_swdge_queues=4)

    xT = nc.dram_tensor("xT", [IN_F, J], BF16, kind="ExternalInput")
    idx16 = nc.dram_tensor("idx16", [P, TOTC], I16, kind="ExternalInput")
    Ball = nc.dram_tensor("Ball", [P, SBTOT, P], FP8, kind="ExternalInput")
    degrecip = nc.dram_tensor("degrecip", [P, G], F32, kind="ExternalInput")
    A1 = nc.dram_tensor("A1", [IN_F, HID_F], BF16, kind="ExternalInput")
    B1 = nc.dram_tensor("B1", [IN_F, HID_F], BF16, kind="ExternalInput")
    C1 = nc.dram_tensor("C1", [IN_F, HID_F], BF16, kind="ExternalInput")
    A2 = nc.dram_tensor("A2", [HID_F, OUT_F], BF16, kind="ExternalInput")
    B2 = nc.dram_tensor("B2", [HID_F, OUT_F], BF16, kind="ExternalInput")
    C2 = nc.dram_tensor("C2", [HID_F, OUT_F], BF16, kind="ExternalInput")
    b1b = nc.dram_tensor("b1b", [P, HID_F], F32, kind="ExternalInput")
    b2b = nc.dram_tensor("b2b", [P, OUT_F], F32, kind="ExternalInput")
    ident = nc.dram_tensor("ident", [P, P], F32, kind="ExternalInput")
    y = nc.dram_tensor("y", [J, OUT_F], F32, kind="ExternalOutput")

    h1_in = []
    h1_tbl = []
    h2_in = []
    h2_tbl = []
    for p in range(NCH):
        rp = PIECE_ROWS[p]
        h1_in.append(nc.dram_tensor(f"h1_in{p}", [rp, HID_F], BF16,
                                    kind="Internal"))
        h1_tbl.append(nc.dram_tensor(f"h1_tbl{p}", [K * rp, HID_F], BF16,
                                     kind="Internal", addr_space="Shared"))
        h2_in.append(nc.dram_tensor(f"h2_in{p}", [rp, OUT_F], F32,
                                    kind="Internal"))
        h2_tbl.append(nc.dram_tensor(f"h2_tbl{p}", [K * rp, OUT_F], F32,
                                     kind="Internal", addr_space="Shared"))

    # pieces per window: (chunk, padded-stream offset, n)
    pieces_w = []
    win_sb0 = []
    win_sb = []
    for w in range(G):
        pieces = []
        for c in range(NCH):
            L = int(Lpad[w, c])
            off = int(run_off[w, c])
            o = 0
            while o < L:
                n = min(L - o, RMAX * P)
                pieces.append((c, off + o, n))
                o += n
        pieces_w.append(pieces)
        win_sb0.append(int(run_off[w, 0]) // P)
        win_sb.append(sum(int(Lpad[w, c]) for c in range(NCH)) // P)
    MAXSB = max(win_sb)

    qctr = [0]

    def next_q():
        q = qctr[0] % 4
        qctr[0] += 1
        return q

    def piece_end_w(w):
        for p in range(NCH):
            if w == PIECE_W0[p] + PIECE_W[p] - 1:
                return p
        return None

    with tile.TileContext(nc) as tc:
        with (
            tc.tile_pool(name="consts", bufs=1) as cpool,
            tc.tile_pool(name="work", bufs=3) as wpool,
            tc.tile_pool(name="gath", bufs=5) as gpool,
            tc.tile_pool(name="m16p", bufs=3) as mpool,
            tc.tile_pool(name="bmat", bufs=6) as bpool,
            tc.tile_pool(name="psum", bufs=2, space="PSUM") as ppool,
        ):
            c_idx = cpool.tile([P, TOTC], I16, tag="idx16")
            nc.sync.dma_start(out=c_idx[:], in_=idx16[:, :])
            c_dr = cpool.tile([P, G], F32, tag="dr")
            nc.sync.dma_start(out=c_dr[:], in_=degrecip[:, :])
            c_w1 = []
            for nm, t in (("A1", A1), ("B1", B1), ("C1", C1)):
                wt = cpool.tile([IN_F, HID_F], BF16, tag=nm)
                nc.sync.dma_start(out=wt[:], in_=t[:, :])
                c_w1.append(wt)
            c_w2 = []
            for nm, t in (("A2", A2), ("B2", B2), ("C2", C2)):
                wt = cpool.tile([HID_F, OUT_F], BF16, tag=nm)
                nc.sync.dma_start(out=wt[:], in_=t[:, :])
                c_w2.append(wt)
            c_b1 = cpool.tile([P, HID_F], F32, tag="b1b")
            nc.sync.dma_start(out=c_b1[:], in_=b1b[:, :])
            c_b2 = cpool.tile([P, OUT_F], F32, tag="b2b")
            nc.sync.dma_start(out=c_b2[:], in_=b2b[:, :])
            c_id = cpool.tile([P, P], F32, tag="ident")
            nc.sync.dma_start(out=c_id[:], in_=ident[:, :])
            # per-window softmax state, filled in phase 3
            tn_all = cpool.tile([P, G, OUT_F], F32, tag="tn_all")
            nmx_all = cpool.tile([P, G], F32, tag="nmx_all")
            se_all = cpool.tile([P, G], F32, tag="se_all")

            # ---------------- phase 1: KAN layer 1 on the shard ----------
            for w in range(G):
                pc = int(_PIECE_OF_W[w])
                r0 = (w - PIECE_W0[pc]) * P
                xt = wpool.tile([IN_F, P], BF16, tag="xt")
                nc.sync.dma_start(out=xt[:], in_=xT[:, w * P:(w + 1) * P])
                x2 = wpool.tile([IN_F, P], BF16, tag="x2")
                nc.vector.tensor_tensor(out=x2[:], in0=xt[:], in1=xt[:],
                                        op=mybir.AluOpType.mult)
                x3 = wpool.tile([IN_F, P], BF16, tag="x3")
                nc.vector.tensor_tensor(out=x3[:], in0=x2[:], in1=xt[:],
                                        op=mybir.AluOpType.mult)
                ps = ppool.tile([P, HID_F], F32, tag="big")
                nc.tensor.matmul(out=ps[:], lhsT=xt[:], rhs=c_w1[0][:],
                                 start=True, stop=False)
                nc.tensor.matmul(out=ps[:], lhsT=x2[:], rhs=c_w1[1][:],
                                 start=False, stop=False)
                nc.tensor.matmul(out=ps[:], lhsT=x3[:], rhs=c_w1[2][:],
                                 start=False, stop=True)
                hb = wpool.tile([P, HID_F], F32, tag="hb")
                nc.vector.tensor_tensor(out=hb[:], in0=ps[:], in1=c_b1[:],
                                        op=mybir.AluOpType.add)
                h1t = wpool.tile([P, HID_F], BF16, tag="h1t")
                nc.vector.tensor_scalar_max(h1t[:], hb[:], 0.0)
                nc.sync.dma_start(out=h1_in[pc][r0:r0 + P, :], in_=h1t[:])
                pe = piece_end_w(w)
                if pe is not None:
                    nc.gpsimd.collective_compute(
                        "AllGather", mybir.AluOpType.bypass,
                        replica_groups=[list(range(K))],
                        ins=[h1_in[pe][:, :]], outs=[h1_tbl[pe][:, :]],
                    )

            # ---------------- phase 2: aggregate + KAN layer 2 -----------
            # 4 windows in flight, one SWDGE queue per window
            for wg in range(0, G, 4):
                ws = list(range(wg, min(wg + 4, G)))
                handles = {}
                for w in ws:
                    q = w % 4
                    bt = bpool.tile([P, MAXSB, P], FP8, tag="b")
                    nc.scalar.dma_start(
                        out=bt[:, :win_sb[w], :],
                        in_=Ball[:, win_sb0[w]:win_sb0[w] + win_sb[w], :])
                    gts = []
                    for (c, poff, n) in pieces_w[w]:
                        nsb = n // P
                        gt = gpool.tile([P, RMAX, HID_F], BF16,
                                        tag=f"g1q{q}")
                        nc.gpsimd.dma_gather(
                            gt[:, :nsb, :],
                            h1_tbl[c][:, :],
                            c_idx[:, poff // 16:(poff + n) // 16],
                            n, n, HID_F, queue_num=q)
                        gts.append((gt, poff, n))
                    handles[w] = (bt, gts)
                for w in ws:
                    bt, gts = handles[w]
                    pbin = ppool.tile([P, HID_F], F32, tag="big")
                    nsb_tot = win_sb[w]
                    si = 0
                    for (gt, poff, n) in gts:
                        nsb = n // P
                        sb0 = poff // P - win_sb0[w]
                        for s in range(nsb):
                            nc.tensor.matmul(
                                out=pbin[:], lhsT=bt[:, sb0 + s, :],
                                rhs=gt[:, s, :],
                                start=(si == 0), stop=(si == nsb_tot - 1))
                            si += 1
                    agg = wpool.tile([P, HID_F], F32, tag="agg")
                    nc.vector.tensor_scalar_mul(agg[:], pbin[:],
                                                c_dr[:, w:w + 1])
                    pt = ppool.tile([P, P], F32, tag="tr")
                    nc.tensor.transpose(out=pt[:], in_=agg[:],
                                        identity=c_id[:])
                    hT = wpool.tile([HID_F, P], BF16, tag="hT")
                    nc.vector.tensor_scalar_mul(hT[:], pt[:], 1.0)
                    q2 = wpool.tile([HID_F, P], BF16, tag="q2")
                    nc.vector.tensor_tensor(out=q2[:], in0=hT[:], in1=hT[:],
                                            op=mybir.AluOpType.mult)
                    q3 = wpool.tile([HID_F, P], BF16, tag="q3")
                    nc.vector.tensor_tensor(out=q3[:], in0=q2[:], in1=hT[:],
                                            op=mybir.AluOpType.mult)
                    ps2 = ppool.tile([P, OUT_F], F32, tag="small")
                    nc.tensor.matmul(out=ps2[:], lhsT=hT[:], rhs=c_w2[0][:],
                                     start=True, stop=False)
                    nc.tensor.matmul(out=ps2[:], lhsT=q2[:], rhs=c_w2[1][:],
                                     start=False, stop=False)
                    nc.tensor.matmul(out=ps2[:], lhsT=q3[:], rhs=c_w2[2][:],
                                     start=False, stop=True)
                    hb2 = wpool.tile([P, OUT_F], F32, tag="hb2")
                    nc.vector.tensor_tensor(out=hb2[:], in0=ps2[:],
                                            in1=c_b2[:],
                                            op=mybir.AluOpType.add)
                    pc = int(_PIECE_OF_W[w])
                    r0 = (w - PIECE_W0[pc]) * P
                    nc.scalar.dma_start(out=h2_in[pc][r0:r0 + P, :],
                                        in_=hb2[:])
                    pe = piece_end_w(w)
                    if pe is not None:
                        nc.gpsimd.collective_compute(
                            "AllGather", mybir.AluOpType.bypass,
                            replica_groups=[list(range(K))],
                            ins=[h2_in[pe][:, :]], outs=[h2_tbl[pe][:, :]],
                        )

            # ---------------- phase 3: aggregate + log_softmax -----------
            # 4 windows in flight, one SWDGE queue per window
            for wg in range(0, G, 4):
                ws = list(range(wg, min(wg + 4, G)))
                handles = {}
                for w in ws:
                    q = w % 4
                    bt = bpool.tile([P, MAXSB, P], FP8, tag="b")
                    nc.scalar.dma_start(
                        out=bt[:, :win_sb[w], :],
                        in_=Ball[:, win_sb0[w]:win_sb0[w] + win_sb[w], :])
                    gts = []
                    for (c, poff, n) in pieces_w[w]:
                        nsb = n // P
                        gt = gpool.tile([P, RMAX, OUT_F], F32, tag=f"g2q{q}")
                        nc.gpsimd.dma_gather(
                            gt[:, :nsb, :],
                            h2_tbl[c][:, :],
                            c_idx[:, poff // 16:(poff + n) // 16],
                            n, n, OUT_F, queue_num=q)
                        m16 = mpool.tile([P, RMAX, OUT_F], BF16,
                                         tag=f"m16q{q}")
                        nc.vector.tensor_scalar_mul(m16[:, :nsb, :],
                                                    gt[:, :nsb, :], 1.0)
                        gts.append((m16, poff, n))
                    handles[w] = (bt, gts)
                for w in ws:
                    bt, gts = handles[w]
                    pb3 = ppool.tile([P, OUT_F], F32, tag="small")
                    nsb_tot = win_sb[w]
                    si = 0
                    for (m16, poff, n) in gts:
                        nsb = n // P
                        sb0 = poff // P - win_sb0[w]
                        for s in range(nsb):
                            nc.tensor.matmul(
                                out=pb3[:], lhsT=bt[:, sb0 + s, :],
                                rhs=m16[:, s, :],
                                start=(si == 0), stop=(si == nsb_tot - 1))
                            si += 1
                    nc.vector.tensor_scalar_mul(tn_all[:, w, :], pb3[:],
                                                c_dr[:, w:w + 1])
                    mx = wpool.tile([P, 1], F32, tag="mx")
                    nc.vector.tensor_reduce(out=mx[:], in_=tn_all[:, w, :],
                                            axis=mybir.AxisListType.X,
                                            op=mybir.AluOpType.max)
                    nc.vector.tensor_scalar_mul(nmx_all[:, w:w + 1], mx[:],
                                                -1.0)
                    et = wpool.tile([P, OUT_F], F32, tag="et")
                    nc.scalar.activation(
                        out=et[:], in_=tn_all[:, w, :],
                        func=mybir.ActivationFunctionType.Exp,
                        bias=nmx_all[:, w:w + 1], scale=1.0,
                        accum_out=se_all[:, w:w + 1])
            lse_all = cpool.tile([P, G], F32, tag="lse_all")
            nc.scalar.activation(out=lse_all[:], in_=se_all[:],
                                 func=mybir.ActivationFunctionType.Ln)
            for w in range(G):
                ot = wpool.tile([P, OUT_F], F32, tag="ot")
                nc.vector.tensor_scalar(ot[:], tn_all[:, w, :],
                                        nmx_all[:, w:w + 1],
                                        lse_all[:, w:w + 1],
                                        mybir.AluOpType.add,
                                        mybir.AluOpType.subtract)
                nc.sync.dma_start(out=y[w * P:(w + 1) * P, :], in_=ot[:])

    nc.compile()
    return nc


def kernel(x, edge_index, w1, b1, c1, w2, b2, c2):
    in_maps, meta = _host_prep(x, edge_index, w1, b1, c1, w2, b2, c2)
    nc = build_program(meta)
    res = run_bass_kernel_spmd(nc, in_maps, core_ids=list(range(K)))
    order = meta["order"]
    out = np.empty((N_NODES, OUT_F), dtype=np.float32)
    jr = np.arange(JREAL)
    for k in range(K):
        out[order[jr * K + k]] = res.results[k]["y"][:JREAL]
    return out


# revision 14
# speedup vs baseline: 1.1660x; 1.1660x over previous
"""KAN-GNN message passing on 8 TRN2 NeuronCores.

Strategy (data-parallel over nodes, per sharding hint):
 - Nodes ranked by in-degree, dealt round-robin to 8 cores (12544 local
   targets each, 98 windows of 128). The gathered tables are built as 4
   piece-wise AllGathers (windows 0-24, 25-49, 50-73, 74-97) so (a) each
   piece is <=25600 rows, addressable by the hardware dma_gather's int16
   indices, and (b) each collective overlaps with compute that produced
   or consumes the neighbouring pieces.
 - Phase 1: KAN layer 1 per local node (3 fused matmuls, bias+relu on
   DVE); AllGather piece p fires as soon as its windows are written.
 - Aggregation is edge-major: per core, in-edges sorted by (target
   window, source piece); each (w,c) run padded to a multiple of 128
   (shared across cores for SPMD) and fetched with hardware dma_gather
   (<=1024 rows per instruction, 4 SWDGE queues round-robin). A
   per-sub-batch 0/1 "binning" matrix B (fp8, streamed per window on the
   ACT engine's HWDGE) scatters each 128 gathered messages onto the
   window's 128 targets via one PE matmul accumulating in PSUM; the
   drain applies 1/deg on DVE.
 - KAN layer 2 per window (PE transpose, DVE powers, 3 matmuls); h2 kept
   f32 (256B rows) and AllGathered piece-wise during phase 2; second
   edge-major gather + binning pass; log_softmax with deferred Ln so the
   ACT engine loads each activation table once.
 - All indices/B matrices precomputed on host; per-core data as inputs.
"""
import numpy as np
import ml_dtypes

import concourse.bacc as bacc
import concourse.mybir as mybir
import concourse.tile as tile
import concourse.bass as bass
from concourse.bass_utils import run_bass_kernel_spmd

N_NODES = 100000
N_EDGES = 1600000
IN_F, HID_F, OUT_F = 128, 128, 64
K = 8               # cores
P = 128             # partitions
J = 12544           # local nodes per core (98*128), 12500 real + 44 pad
G = J // P          # 98 windows
JREAL = N_NODES // K
NCH = 4             # table pieces / gather chunks
PIECE_W = [25, 25, 24, 24]          # windows per piece
PIECE_W0 = [0, 25, 50, 74]          # first window of each piece
PIECE_ROWS = [w * P for w in PIECE_W]   # local rows per piece
RMAX = 8            # max sub-batches (of 128 edges) per gather piece

BF16 = mybir.dt.bfloat16
F32 = mybir.dt.float32
I16 = mybir.dt.int16
FP8 = mybir.dt.float8e4

_PIECE_OF_W = np.repeat(np.arange(NCH), PIECE_W)


def _host_prep(x, edge_index, w1, b1, c1, w2, b2, c2):
    src = np.asarray(edge_index[0], dtype=np.int64)
    tgt = np.asarray(edge_index[1], dtype=np.int64)
    x = np.asarray(x, dtype=np.float32)

    deg = np.bincount(tgt, minlength=N_NODES)
    order = np.argsort(-deg, kind="stable")
    rank_of = np.empty(N_NODES, dtype=np.int64)
    rank_of[order] = np.arange(N_NODES)
    core_of = rank_of % K
    j_of = rank_of // K

    # table position within its piece: rel = core*rows_p + (j - 128*w0_p)
    w_of = j_of // P
    piece_of = _PIECE_OF_W[w_of]
    rows_p = np.array(PIECE_ROWS)[piece_of]
    w0_p = np.array(PIECE_W0)[piece_of]
    rel_of = core_of * rows_p + (j_of - w0_p * P)

    # per-edge attributes
    ek = core_of[tgt]                    # owning core (by target)
    we = j_of[tgt] // P                  # target window
    ce = piece_of[src]                   # source chunk/piece
    rel = rel_of[src]                    # in-piece table row
    tcol = j_of[tgt] % P                 # target column within window

    key = (ek * G + we) * NCH + ce
    eorder = np.lexsort((rel, key))
    skey = key[eorder]
    counts = np.bincount(key, minlength=K * G * NCH).reshape(K, G, NCH)
    Lmax = counts.max(axis=0)                        # [G, NCH]
    Lpad = ((Lmax + P - 1) // P) * P                 # shared padded run lens

    run_off = np.zeros((G, NCH), dtype=np.int64)
    TOT = 0
    for w in range(G):
        for c in range(NCH):
            run_off[w, c] = TOT
            TOT += int(Lpad[w, c])
    SBTOT = TOT // P

    # place each core's sorted edges into the shared padded stream
    flat_counts = counts.reshape(-1)
    run_starts_e = np.concatenate([[0], np.cumsum(flat_counts)[:-1]])
    d_in_run = np.arange(len(skey)) - np.repeat(run_starts_e, flat_counts)
    kk = skey // (G * NCH)
    ww = (skey // NCH) % G
    cc = skey % NCH
    ppos = run_off[ww, cc] + d_in_run

    idx_rel = np.zeros((K, TOT), dtype=np.int16)     # pads -> row 0 (B=0)
    tcol_pad = np.full((K, TOT), -1, dtype=np.int64)
    idx_rel[kk, ppos] = rel[eorder].astype(np.int16)
    tcol_pad[kk, ppos] = tcol[eorder]

    # int16 index stream wrapped in 16 partitions, replicated x8 gpsimd cores
    blk = idx_rel.reshape(K, TOT // 16, 16).transpose(0, 2, 1)
    idx16 = np.ascontiguousarray(np.tile(blk, (1, 8, 1)))  # [K, 128, TOT//16]

    # binning matrices: Ball[k, p, s, t] = 1 if edge (s*128+p) targets t
    bm = np.zeros((K, TOT, P), dtype=ml_dtypes.float8_e4m3)
    kidx, eidx = np.nonzero(tcol_pad >= 0)
    bm[kidx, eidx, tcol_pad[kidx, eidx]] = 1.0
    Ball = np.ascontiguousarray(
        bm.reshape(K, SBTOT, P, P).transpose(0, 2, 1, 3))  # [K,128,SBTOT,128]

    # 1/deg per local target [K, P, G] (0 for pad targets)
    degs_kj = np.zeros((K, J), dtype=np.int64)
    degs_kj[core_of, j_of] = deg
    dr = 1.0 / np.maximum(degs_kj, 1).astype(np.float32)
    real = np.zeros((K, J), dtype=np.float32)
    real[:, :JREAL] = 1.0
    degrecip = np.ascontiguousarray(
        (dr * real).reshape(K, G, P).transpose(0, 2, 1))

    # xT shards, bf16 [K][IN_F, J]
    xT = np.zeros((K, IN_F, J), dtype=ml_dtypes.bfloat16)
    for k in range(K):
        nodes_k = order[np.arange(JREAL) * K + k]
        xT[k, :, :JREAL] = x[nodes_k].T.astype(ml_dtypes.bfloat16)

    # fused KAN weights
    A1 = (w1 + 0.1 * c1[:, :, 0]).astype(ml_dtypes.bfloat16)
    B1 = (0.1 * c1[:, :, 1]).astype(ml_dtypes.bfloat16)
    C1 = (0.1 * c1[:, :, 2]).astype(ml_dtypes.bfloat16)
    A2 = (w2 + 0.1 * c2[:, :, 0]).astype(ml_dtypes.bfloat16)
    B2 = (0.1 * c2[:, :, 1]).astype(ml_dtypes.bfloat16)
    C2 = (0.1 * c2[:, :, 2]).astype(ml_dtypes.bfloat16)
    b1b = np.tile(np.asarray(b1, np.float32)[None, :], (P, 1))
    b2b = np.tile(np.asarray(b2, np.float32)[None, :], (P, 1))
    ident = np.eye(P, dtype=np.float32)

    in_maps = []
    for k in range(K):
        in_maps.append({
            "xT": xT[k],
            "idx16": idx16[k],
            "Ball": Ball[k],
            "degrecip": degrecip[k],
            "A1": A1, "B1": B1, "C1": C1,
            "A2": A2, "B2": B2, "C2": C2,
            "b1b": b1b, "b2b": b2b, "ident": ident,
        })
    meta = {"Lpad": Lpad, "run_off": run_off, "TOT": TOT, "SBTOT": SBTOT,
            "order": order}
    return in_maps, meta


def build_program(meta):
    Lpad = meta["Lpad"]
    run_off = meta["run_off"]
    TOT = int(meta["TOT"])
    SBTOT = int(meta["SBTOT"])
    TOTC = TOT // 16

    nc = bacc.Bacc("TRN2", target_bir_lowering=False, debug=False, num_devices=K,
                   num_swdge_queues=4)

    xT = nc.dram_tensor("xT", [IN_F, J], BF16, kind="ExternalInput")
    idx16 = nc.dram_tensor("idx16", [P, TOTC], I16, kind="ExternalInput")
    Ball = nc.dram_tensor("Ball", [P, SBTOT, P], FP8, kind="ExternalInput")
    degrecip = nc.dram_tensor("degrecip", [P, G], F32, kind="ExternalInput")
    A1 = nc.dram_tensor("A1", [IN_F, HID_F], BF16, kind="ExternalInput")
    B1 = nc.dram_tensor("B1", [IN_F, HID_F], BF16, kind="ExternalInput")
    C1 = nc.dram_tensor("C1", [IN_F, HID_F], BF16, kind="ExternalInput")
    A2 = nc.dram_tensor("A2", [HID_F, OUT_F], BF16, kind="ExternalInput")
    B2 = nc.dram_tensor("B2", [HID_F, OUT_F], BF16, kind="ExternalInput")
    C2 = nc.dram_tensor("C2", [HID_F, OUT_F], BF16, kind="ExternalInput")
    b1b = nc.dram_tensor("b1b", [P, HID_F], F32, kind="ExternalInput")
    b2b = nc.dram_tensor("b2b", [P, OUT_F], F32, kind="ExternalInput")
    ident = nc.dram_tensor("ident", [P, P], F32, kind="ExternalInput")
    y = nc.dram_tensor("y", [J, OUT_F], F32, kind="ExternalOutput")

    h1_in = []
    h1_tbl = []
    h2_in = []
    h2_tbl = []
    for p in range(NCH):
        rp = PIECE_ROWS[p]
        h1_in.append(nc.dram_tensor(f"h1_in{p}", [rp, HID_F], BF16,
                                    kind="Internal"))
        h1_tbl.append(nc.dram_tensor(f"h1_tbl{p}", [K * rp, HID_F], BF16,
                                     kind="Internal", addr_space="Shared"))
        h2_in.append(nc.dram_tensor(f"h2_in{p}", [rp, OUT_F], F32,
                                    kind="Internal"))
        h2_tbl.append(nc.dram_tensor(f"h2_tbl{p}", [K * rp, OUT_F], F32,
                                     kind="Internal", addr_space="Shared"))

    # pieces per window: (chunk, padded-stream offset, n)
    pieces_w = []
    win_sb0 = []
    win_sb = []
    for w in range(G):
        pieces = []
        for c in range(NCH):
            L = int(Lpad[w, c])
            off = int(run_off[w, c])
            o = 0
            while o < L:
                n = min(L - o, RMAX * P)
                pieces.append((c, off + o, n))
                o += n
        pieces_w.append(pieces)
        win_sb0.append(int(run_off[w, 0]) // P)
        win_sb.append(sum(int(Lpad[w, c]) for c in range(NCH)) // P)
    MAXSB = max(win_sb)

    qctr = [0]

    def next_q():
        q = qctr[0] % 4
        qctr[0] += 1
        return q

    def piece_end_w(w):
        for p in range(NCH):
            if w == PIECE_W0[p] + PIECE_W[p] - 1:
                return p
        return None

    with tile.TileContext(nc) as tc:
        with (
            tc.tile_pool(name="consts", bufs=1) as cpool,
            tc.tile_pool(name="work", bufs=3) as wpool,
            tc.tile_pool(name="gath", bufs=16) as gpool,
            tc.tile_pool(name="m16p", bufs=6) as mpool,
            tc.tile_pool(name="bmat", bufs=5) as bpool,
            tc.tile_pool(name="psum", bufs=2, space="PSUM") as ppool,
        ):
            c_idx = cpool.tile([P, TOTC], I16, tag="idx16")
            nc.sync.dma_start(out=c_idx[:], in_=idx16[:, :])
            c_dr = cpool.tile([P, G], F32, tag="dr")
            nc.sync.dma_start(out=c_dr[:], in_=degrecip[:, :])
            c_w1 = []
            for nm, t in (("A1", A1), ("B1", B1), ("C1", C1)):
                wt = cpool.tile([IN_F, HID_F], BF16, tag=nm)
                nc.sync.dma_start(out=wt[:], in_=t[:, :])
                c_w1.append(wt)
            c_w2 = []
            for nm, t in (("A2", A2), ("B2", B2), ("C2", C2)):
                wt = cpool.tile([HID_F, OUT_F], BF16, tag=nm)
                nc.sync.dma_start(out=wt[:], in_=t[:, :])
                c_w2.append(wt)
            c_b1 = cpool.tile([P, HID_F], F32, tag="b1b")
            nc.sync.dma_start(out=c_b1[:], in_=b1b[:, :])
            c_b2 = cpool.tile([P, OUT_F], F32, tag="b2b")
            nc.sync.dma_start(out=c_b2[:], in_=b2b[:, :])
            c_id = cpool.tile([P, P], F32, tag="ident")
            nc.sync.dma_start(out=c_id[:], in_=ident[:, :])
            # per-window softmax state, filled in phase 3
            tn_all = cpool.tile([P, G, OUT_F], F32, tag="tn_all")
            nmx_all = cpool.tile([P, G], F32, tag="nmx_all")
            se_all = cpool.tile([P, G], F32, tag="se_all")

            # ---------------- phase 1: KAN layer 1 on the shard ----------
            for w in range(G):
                pc = int(_PIECE_OF_W[w])
                r0 = (w - PIECE_W0[pc]) * P
                xt = wpool.tile([IN_F, P], BF16, tag="xt")
                nc.sync.dma_start(out=xt[:], in_=xT[:, w * P:(w + 1) * P])
                x2 = wpool.tile([IN_F, P], BF16, tag="x2")
                nc.vector.tensor_tensor(out=x2[:], in0=xt[:], in1=xt[:],
                                        op=mybir.AluOpType.mult)
                x3 = wpool.tile([IN_F, P], BF16, tag="x3")
                nc.vector.tensor_tensor(out=x3[:], in0=x2[:], in1=xt[:],
                                        op=mybir.AluOpType.mult)
                ps = ppool.tile([P, HID_F], F32, tag="big")
                nc.tensor.matmul(out=ps[:], lhsT=xt[:], rhs=c_w1[0][:],
                                 start=True, stop=False)
                nc.tensor.matmul(out=ps[:], lhsT=x2[:], rhs=c_w1[1][:],
                                 start=False, stop=False)
                nc.tensor.matmul(out=ps[:], lhsT=x3[:], rhs=c_w1[2][:],
                                 start=False, stop=True)
                hb = wpool.tile([P, HID_F], F32, tag="hb")
                nc.vector.tensor_tensor(out=hb[:], in0=ps[:], in1=c_b1[:],
                                        op=mybir.AluOpType.add)
                h1t = wpool.tile([P, HID_F], BF16, tag="h1t")
                nc.vector.tensor_scalar_max(h1t[:], hb[:], 0.0)
                nc.sync.dma_start(out=h1_in[pc][r0:r0 + P, :], in_=h1t[:])
                pe = piece_end_w(w)
                if pe is not None:
                    nc.gpsimd.collective_compute(
                        "AllGather", mybir.AluOpType.bypass,
                        replica_groups=[list(range(K))],
                        ins=[h1_in[pe][:, :]], outs=[h1_tbl[pe][:, :]],
                    )

            # ---------------- phase 2: aggregate + KAN layer 2 -----------
            # software-pipelined: gathers/B loads issued LOOK windows ahead
            LOOK = 3
            inflight = {}

            def emit2(w):
                bt = bpool.tile([P, MAXSB, P], FP8, tag="b")
                nc.scalar.dma_start(
                    out=bt[:, :win_sb[w], :],
                    in_=Ball[:, win_sb0[w]:win_sb0[w] + win_sb[w], :])
                gts = []
                for (c, poff, n) in pieces_w[w]:
                    nsb = n // P
                    gt = gpool.tile([P, RMAX, HID_F], BF16, tag="g1")
                    nc.gpsimd.dma_gather(
                        gt[:, :nsb, :],
                        h1_tbl[c][:, :],
                        c_idx[:, poff // 16:(poff + n) // 16],
                        n, n, HID_F, queue_num=next_q())
                    gts.append((gt, poff, n))
                inflight[w] = (bt, gts)

            for w in range(min(LOOK, G)):
                emit2(w)
            for w in range(G):
                if w + LOOK < G:
                    emit2(w + LOOK)
                bt, gts = inflight.pop(w)
                pbin = ppool.tile([P, HID_F], F32, tag="big")
                nsb_tot = win_sb[w]
                si = 0
                for (gt, poff, n) in gts:
                    nsb = n // P
                    sb0 = poff // P - win_sb0[w]
                    for s in range(nsb):
                        nc.tensor.matmul(
                            out=pbin[:], lhsT=bt[:, sb0 + s, :],
                            rhs=gt[:, s, :],
                            start=(si == 0), stop=(si == nsb_tot - 1))
                        si += 1
                agg = wpool.tile([P, HID_F], F32, tag="agg")
                nc.vector.tensor_scalar_mul(agg[:], pbin[:], c_dr[:, w:w + 1])
                pt = ppool.tile([P, P], F32, tag="tr")
                nc.tensor.transpose(out=pt[:], in_=agg[:], identity=c_id[:])
                hT = wpool.tile([HID_F, P], BF16, tag="hT")
                nc.vector.tensor_scalar_mul(hT[:], pt[:], 1.0)
                q2 = wpool.tile([HID_F, P], BF16, tag="q2")
                nc.vector.tensor_tensor(out=q2[:], in0=hT[:], in1=hT[:],
                                        op=mybir.AluOpType.mult)
                q3 = wpool.tile([HID_F, P], BF16, tag="q3")
                nc.vector.tensor_tensor(out=q3[:], in0=q2[:], in1=hT[:],
                                        op=mybir.AluOpType.mult)
                ps2 = ppool.tile([P, OUT_F], F32, tag="small")
                nc.tensor.matmul(out=ps2[:], lhsT=hT[:], rhs=c_w2[0][:],
                                 start=True, stop=False)
                nc.tensor.matmul(out=ps2[:], lhsT=q2[:], rhs=c_w2[1][:],
                                 start=False, stop=False)
                nc.tensor.matmul(out=ps2[:], lhsT=q3[:], rhs=c_w2[2][:],
                                 start=False, stop=True)
                hb2 = wpool.tile([P, OUT_F], F32, tag="hb2")
                nc.vector.tensor_tensor(out=hb2[:], in0=ps2[:], in1=c_b2[:],
                                        op=mybir.AluOpType.add)
                pc = int(_PIECE_OF_W[w])
                r0 = (w - PIECE_W0[pc]) * P
                nc.scalar.dma_start(out=h2_in[pc][r0:r0 + P, :], in_=hb2[:])
                pe = piece_end_w(w)
                if pe is not None:
                    nc.gpsimd.collective_compute(
                        "AllGather", mybir.AluOpType.bypass,
                        replica_groups=[list(range(K))],
                        ins=[h2_in[pe][:, :]], outs=[h2_tbl[pe][:, :]],
                    )

            # ---------------- phase 3: aggregate + log_softmax -----------
            def emit3(w):
                bt = bpool.tile([P, MAXSB, P], FP8, tag="b")
                nc.scalar.dma_start(
                    out=bt[:, :win_sb[w], :],
                    in_=Ball[:, win_sb0[w]:win_sb0[w] + win_sb[w], :])
                gts = []
                for (c, poff, n) in pieces_w[w]:
                    nsb = n // P
                    gt = gpool.tile([P, RMAX, OUT_F], F32, tag="g2")
                    nc.gpsimd.dma_gather(
                        gt[:, :nsb, :],
                        h2_tbl[c][:, :],
                        c_idx[:, poff // 16:(poff + n) // 16],
                        n, n, OUT_F, queue_num=next_q())
                    gts.append((gt, poff, n))
                inflight[w] = (bt, gts)

            for w in range(min(LOOK, G)):
                emit3(w)
            for w in range(G):
                if w + LOOK < G:
                    emit3(w + LOOK)
                bt, gts = inflight.pop(w)
                pb3 = ppool.tile([P, OUT_F], F32, tag="small")
                nsb_tot = win_sb[w]
                si = 0
                for (gt, poff, n) in gts:
                    nsb = n // P
                    m16 = mpool.tile([P, RMAX, OUT_F], BF16, tag="m16")
                    nc.vector.tensor_scalar_mul(m16[:, :nsb, :],
                                                gt[:, :nsb, :], 1.0)
                    sb0 = poff // P - win_sb0[w]
                    for s in range(nsb):
                        nc.tensor.matmul(
                            out=pb3[:], lhsT=bt[:, sb0 + s, :],
                            rhs=m16[:, s, :],
                            start=(si == 0), stop=(si == nsb_tot - 1))
                        si += 1
                nc.vector.tensor_scalar_mul(tn_all[:, w, :], pb3[:],
                                            c_dr[:, w:w + 1])
                mx = wpool.tile([P, 1], F32, tag="mx")
                nc.vector.tensor_reduce(out=mx[:], in_=tn_all[:, w, :],
                                        axis=mybir.AxisListType.X,
                                        op=mybir.AluOpType.max)
                nc.vector.tensor_scalar_mul(nmx_all[:, w:w + 1], mx[:], -1.0)
                et = wpool.tile([P, OUT_F], F32, tag="et")
                nc.scalar.activation(out=et[:], in_=tn_all[:, w, :],
                                     func=mybir.ActivationFunctionType.Exp,
                                     bias=nmx_all[:, w:w + 1], scale=1.0,
                                     accum_out=se_all[:, w:w + 1])
            lse_all = cpool.tile([P, G], F32, tag="lse_all")
            nc.scalar.activation(out=lse_all[:], in_=se_all[:],
                                 func=mybir.ActivationFunctionType.Ln)
            for w in range(G):
                ot = wpool.tile([P, OUT_F], F32, tag="ot")
                nc.vector.tensor_scalar(ot[:], tn_all[:, w, :],
                                        nmx_all[:, w:w + 1],
                                        lse_all[:, w:w + 1],
                                        mybir.AluOpType.add,
                                        mybir.AluOpType.subtract)
                nc.sync.dma_start(out=y[w * P:(w + 1) * P, :], in_=ot[:])

    nc.compile()
    return nc


def kernel(x, edge_index, w1, b1, c1, w2, b2, c2):
    in_maps, meta = _host_prep(x, edge_index, w1, b1, c1, w2, b2, c2)
    nc = build_program(meta)
    res = run_bass_kernel_spmd(nc, in_maps, core_ids=list(range(K)))
    order = meta["order"]
    out = np.empty((N_NODES, OUT_F), dtype=np.float32)
    jr = np.arange(JREAL)
    for k in range(K):
        out[order[jr * K + k]] = res.results[k]["y"][:JREAL]
    return out


# revision 17
# speedup vs baseline: 1.4240x; 1.2213x over previous
"""KAN-GNN message passing on 8 TRN2 NeuronCores.

Strategy (data-parallel over nodes, per sharding hint):
 - Nodes ranked by in-degree, dealt round-robin to 8 cores (12544 local
   targets each, 98 windows of 128). The gathered tables are built as 4
   piece-wise AllGathers (windows 0-24, 25-49, 50-73, 74-97) so (a) each
   piece is <=25600 rows, addressable by the hardware dma_gather's int16
   indices, and (b) each collective overlaps with compute that produced
   or consumes the neighbouring pieces.
 - Phase 1: KAN layer 1 per local node (3 fused matmuls, bias+relu on
   DVE); AllGather piece p fires as soon as its windows are written.
 - Aggregation is edge-major: per core, in-edges sorted by (target
   window, source piece); each (w,c) run padded to a multiple of 128
   (shared across cores for SPMD) and fetched with hardware dma_gather
   (<=1024 rows per instruction, 4 SWDGE queues round-robin). A
   per-sub-batch 0/1 "binning" matrix B (fp8, streamed per window on the
   ACT engine's HWDGE) scatters each 128 gathered messages onto the
   window's 128 targets via one PE matmul accumulating in PSUM; the
   drain applies 1/deg on DVE.
 - KAN layer 2 per window (PE transpose, DVE powers, 3 matmuls); h2 kept
   f32 (256B rows) and AllGathered piece-wise during phase 2; second
   edge-major gather + binning pass; log_softmax with deferred Ln so the
   ACT engine loads each activation table once.
 - All indices/B matrices precomputed on host; per-core data as inputs.
"""
import numpy as np
import ml_dtypes

import concourse.bacc as bacc
import concourse.mybir as mybir
import concourse.tile as tile
import concourse.bass as bass
from concourse.bass_utils import run_bass_kernel_spmd

N_NODES = 100000
N_EDGES = 1600000
IN_F, HID_F, OUT_F = 128, 128, 64
K = 8               # cores
P = 128             # partitions
J = 12544           # local nodes per core (98*128), 12500 real + 44 pad
G = J // P          # 98 windows
JREAL = N_NODES // K
NCH = 4             # table pieces / gather chunks
PIECE_W = [25, 25, 24, 24]          # windows per piece
PIECE_W0 = [0, 25, 50, 74]          # first window of each piece
PIECE_ROWS = [w * P for w in PIECE_W]   # local rows per piece
RMAX = 8            # max sub-batches (of 128 edges) per gather piece

BF16 = mybir.dt.bfloat16
F32 = mybir.dt.float32
I16 = mybir.dt.int16
FP8 = mybir.dt.float8e4

_PIECE_OF_W = np.repeat(np.arange(NCH), PIECE_W)


def _host_prep(x, edge_index, w1, b1, c1, w2, b2, c2):
    src = np.asarray(edge_index[0], dtype=np.int64)
    tgt = np.asarray(edge_index[1], dtype=np.int64)
    x = np.asarray(x, dtype=np.float32)

    deg = np.bincount(tgt, minlength=N_NODES)
    order = np.argsort(-deg, kind="stable")
    rank_of = np.empty(N_NODES, dtype=np.int64)
    rank_of[order] = np.arange(N_NODES)
    core_of = rank_of % K
    j_of = rank_of // K

    # table position within its piece: rel = core*rows_p + (j - 128*w0_p)
    w_of = j_of // P
    piece_of = _PIECE_OF_W[w_of]
    rows_p = np.array(PIECE_ROWS)[piece_of]
    w0_p = np.array(PIECE_W0)[piece_of]
    rel_of = core_of * rows_p + (j_of - w0_p * P)

    # per-edge attributes
    ek = core_of[tgt]                    # owning core (by target)
    we = j_of[tgt] // P                  # target window
    ce = piece_of[src]                   # source chunk/piece
    rel = rel_of[src]                    # in-piece table row
    tcol = j_of[tgt] % P                 # target column within window

    key = (ek * G + we) * NCH + ce
    eorder = np.lexsort((rel, key))
    skey = key[eorder]
    counts = np.bincount(key, minlength=K * G * NCH).reshape(K, G, NCH)
    Lmax = counts.max(axis=0)                        # [G, NCH]
    Lpad = ((Lmax + P - 1) // P) * P                 # shared padded run lens

    run_off = np.zeros((G, NCH), dtype=np.int64)
    TOT = 0
    for w in range(G):
        for c in range(NCH):
            run_off[w, c] = TOT
            TOT += int(Lpad[w, c])
    SBTOT = TOT // P

    # place each core's sorted edges into the shared padded stream
    flat_counts = counts.reshape(-1)
    run_starts_e = np.concatenate([[0], np.cumsum(flat_counts)[:-1]])
    d_in_run = np.arange(len(skey)) - np.repeat(run_starts_e, flat_counts)
    kk = skey // (G * NCH)
    ww = (skey // NCH) % G
    cc = skey % NCH
    ppos = run_off[ww, cc] + d_in_run

    # pad entries point at consecutive rows 0,1,2,... so the DMA engine can
    # aggregate their descriptors into contiguous reads; B=0 kills the values
    idx_rel = np.tile((np.arange(TOT) % P).astype(np.int16), (K, 1))
    tcol_pad = np.full((K, TOT), -1, dtype=np.int64)
    idx_rel[kk, ppos] = rel[eorder].astype(np.int16)
    tcol_pad[kk, ppos] = tcol[eorder]

    # int16 index stream wrapped in 16 partitions, replicated x8 gpsimd cores
    blk = idx_rel.reshape(K, TOT // 16, 16).transpose(0, 2, 1)
    idx16 = np.ascontiguousarray(np.tile(blk, (1, 8, 1)))  # [K, 128, TOT//16]

    # binning matrices: Ball[k, p, s, t] = 1 if edge (s*128+p) targets t
    bm = np.zeros((K, TOT, P), dtype=ml_dtypes.float8_e4m3)
    kidx, eidx = np.nonzero(tcol_pad >= 0)
    bm[kidx, eidx, tcol_pad[kidx, eidx]] = 1.0
    Ball = np.ascontiguousarray(
        bm.reshape(K, SBTOT, P, P).transpose(0, 2, 1, 3))  # [K,128,SBTOT,128]

    # 1/deg per local target [K, P, G] (0 for pad targets)
    degs_kj = np.zeros((K, J), dtype=np.int64)
    degs_kj[core_of, j_of] = deg
    dr = 1.0 / np.maximum(degs_kj, 1).astype(np.float32)
    real = np.zeros((K, J), dtype=np.float32)
    real[:, :JREAL] = 1.0
    degrecip = np.ascontiguousarray(
        (dr * real).reshape(K, G, P).transpose(0, 2, 1))

    # xT shards, bf16 [K][IN_F, J]
    xT = np.zeros((K, IN_F, J), dtype=ml_dtypes.bfloat16)
    for k in range(K):
        nodes_k = order[np.arange(JREAL) * K + k]
        xT[k, :, :JREAL] = x[nodes_k].T.astype(ml_dtypes.bfloat16)

    # fused KAN weights
    A1 = (w1 + 0.1 * c1[:, :, 0]).astype(ml_dtypes.bfloat16)
    B1 = (0.1 * c1[:, :, 1]).astype(ml_dtypes.bfloat16)
    C1 = (0.1 * c1[:, :, 2]).astype(ml_dtypes.bfloat16)
    A2 = (w2 + 0.1 * c2[:, :, 0]).astype(ml_dtypes.bfloat16)
    B2 = (0.1 * c2[:, :, 1]).astype(ml_dtypes.bfloat16)
    C2 = (0.1 * c2[:, :, 2]).astype(ml_dtypes.bfloat16)
    b1b = np.tile(np.asarray(b1, np.float32)[None, :], (P, 1))
    b2b = np.tile(np.asarray(b2, np.float32)[None, :], (P, 1))
    ident = np.eye(P, dtype=np.float32)

    in_maps = []
    for k in range(K):
        in_maps.append({
            "xT": xT[k],
            "idx16": idx16[k],
            "Ball": Ball[k],
            "degrecip": degrecip[k],
            "A1": A1, "B1": B1, "C1": C1,
            "A2": A2, "B2": B2, "C2": C2,
            "b1b": b1b, "b2b": b2b, "ident": ident,
        })
    meta = {"Lpad": Lpad, "run_off": run_off, "TOT": TOT, "SBTOT": SBTOT,
            "order": order}
    return in_maps, meta


def build_program(meta):
    Lpad = meta["Lpad"]
    run_off = meta["run_off"]
    TOT = int(meta["TOT"])
    SBTOT = int(meta["SBTOT"])
    TOTC = TOT // 16

    nc = bacc.Bacc("TRN2", target_bir_lowering=False, debug=False, num_devices=K,
                   num_swdge_queues=4)

    xT = nc.dram_tensor("xT", [IN_F, J], BF16, kind="ExternalInput")
    idx16 = nc.dram_tensor("idx16", [P, TOTC], I16, kind="ExternalInput")
    Ball = nc.dram_tensor("Ball", [P, SBTOT, P], FP8, kind="ExternalInput")
    degrecip = nc.dram_tensor("degrecip", [P, G], F32, kind="ExternalInput")
    A1 = nc.dram_tensor("A1", [IN_F, HID_F], BF16, kind="ExternalInput")
    B1 = nc.dram_tensor("B1", [IN_F, HID_F], BF16, kind="ExternalInput")
    C1 = nc.dram_tensor("C1", [IN_F, HID_F], BF16, kind="ExternalInput")
    A2 = nc.dram_tensor("A2", [HID_F, OUT_F], BF16, kind="ExternalInput")
    B2 = nc.dram_tensor("B2", [HID_F, OUT_F], BF16, kind="ExternalInput")
    C2 = nc.dram_tensor("C2", [HID_F, OUT_F], BF16, kind="ExternalInput")
    b1b = nc.dram_tensor("b1b", [P, HID_F], F32, kind="ExternalInput")
    b2b = nc.dram_tensor("b2b", [P, OUT_F], F32, kind="ExternalInput")
    ident = nc.dram_tensor("ident", [P, P], F32, kind="ExternalInput")
    y = nc.dram_tensor("y", [J, OUT_F], F32, kind="ExternalOutput")

    h1_in = []
    h1_tbl = []
    h2_in = []
    h2_tbl = []
    for p in range(NCH):
        rp = PIECE_ROWS[p]
        h1_in.append(nc.dram_tensor(f"h1_in{p}", [rp, HID_F], BF16,
                                    kind="Internal"))
        h1_tbl.append(nc.dram_tensor(f"h1_tbl{p}", [K * rp, HID_F], BF16,
                                     kind="Internal", addr_space="Shared"))
        h2_in.append(nc.dram_tensor(f"h2_in{p}", [rp, OUT_F], F32,
                                    kind="Internal"))
        h2_tbl.append(nc.dram_tensor(f"h2_tbl{p}", [K * rp, OUT_F], F32,
                                     kind="Internal", addr_space="Shared"))

    # pieces per window: (chunk, padded-stream offset, n)
    pieces_w = []
    win_sb0 = []
    win_sb = []
    for w in range(G):
        pieces = []
        for c in range(NCH):
            L = int(Lpad[w, c])
            off = int(run_off[w, c])
            o = 0
            while o < L:
                n = min(L - o, RMAX * P)
                pieces.append((c, off + o, n))
                o += n
        pieces_w.append(pieces)
        win_sb0.append(int(run_off[w, 0]) // P)
        win_sb.append(sum(int(Lpad[w, c]) for c in range(NCH)) // P)
    MAXSB = max(win_sb)

    qctr = [0]

    def next_q():
        q = qctr[0] % 4
        qctr[0] += 1
        return q

    def piece_end_w(w):
        for p in range(NCH):
            if w == PIECE_W0[p] + PIECE_W[p] - 1:
                return p
        return None

    with tile.TileContext(nc) as tc:
        with (
            tc.tile_pool(name="consts", bufs=1) as cpool,
            tc.tile_pool(name="work", bufs=3) as wpool,
            tc.tile_pool(name="gath", bufs=16) as gpool,
            tc.tile_pool(name="m16p", bufs=6) as mpool,
            tc.tile_pool(name="bmat", bufs=5) as bpool,
            tc.tile_pool(name="psum", bufs=2, space="PSUM") as ppool,
        ):
            c_idx = cpool.tile([P, TOTC], I16, tag="idx16")
            nc.sync.dma_start(out=c_idx[:], in_=idx16[:, :])
            c_dr = cpool.tile([P, G], F32, tag="dr")
            nc.sync.dma_start(out=c_dr[:], in_=degrecip[:, :])
            c_w1 = []
            for nm, t in (("A1", A1), ("B1", B1), ("C1", C1)):
                wt = cpool.tile([IN_F, HID_F], BF16, tag=nm)
                nc.sync.dma_start(out=wt[:], in_=t[:, :])
                c_w1.append(wt)
            c_w2 = []
            for nm, t in (("A2", A2), ("B2", B2), ("C2", C2)):
                wt = cpool.tile([HID_F, OUT_F], BF16, tag=nm)
                nc.sync.dma_start(out=wt[:], in_=t[:, :])
                c_w2.append(wt)
            c_b1 = cpool.tile([P, HID_F], F32, tag="b1b")
            nc.sync.dma_start(out=c_b1[:], in_=b1b[:, :])
            c_b2 = cpool.tile([P, OUT_F], F32, tag="b2b")
            nc.sync.dma_start(out=c_b2[:], in_=b2b[:, :])
            c_id = cpool.tile([P, P], F32, tag="ident")
            nc.sync.dma_start(out=c_id[:], in_=ident[:, :])
            # per-window softmax state, filled in phase 3
            tn_all = cpool.tile([P, G, OUT_F], F32, tag="tn_all")
            nmx_all = cpool.tile([P, G], F32, tag="nmx_all")
            se_all = cpool.tile([P, G], F32, tag="se_all")

            # ---------------- phase 1: KAN layer 1 on the shard ----------
            for w in range(G):
                pc = int(_PIECE_OF_W[w])
                r0 = (w - PIECE_W0[pc]) * P
                xt = wpool.tile([IN_F, P], BF16, tag="xt")
                nc.sync.dma_start(out=xt[:], in_=xT[:, w * P:(w + 1) * P])
                x2 = wpool.tile([IN_F, P], BF16, tag="x2")
                nc.gpsimd.tensor_tensor(out=x2[:], in0=xt[:], in1=xt[:],
                                        op=mybir.AluOpType.mult)
                x3 = wpool.tile([IN_F, P], BF16, tag="x3")
                nc.gpsimd.tensor_tensor(out=x3[:], in0=x2[:], in1=xt[:],
                                        op=mybir.AluOpType.mult)
                ps = ppool.tile([P, HID_F], F32, tag="big")
                nc.tensor.matmul(out=ps[:], lhsT=xt[:], rhs=c_w1[0][:],
                                 start=True, stop=False)
                nc.tensor.matmul(out=ps[:], lhsT=x2[:], rhs=c_w1[1][:],
                                 start=False, stop=False)
                nc.tensor.matmul(out=ps[:], lhsT=x3[:], rhs=c_w1[2][:],
                                 start=False, stop=True)
                hb = wpool.tile([P, HID_F], F32, tag="hb")
                nc.vector.tensor_tensor(out=hb[:], in0=ps[:], in1=c_b1[:],
                                        op=mybir.AluOpType.add)
                h1t = wpool.tile([P, HID_F], BF16, tag="h1t")
                nc.vector.tensor_scalar_max(h1t[:], hb[:], 0.0)
                nc.sync.dma_start(out=h1_in[pc][r0:r0 + P, :], in_=h1t[:])
                pe = piece_end_w(w)
                if pe is not None:
                    nc.gpsimd.collective_compute(
                        "AllGather", mybir.AluOpType.bypass,
                        replica_groups=[list(range(K))],
                        ins=[h1_in[pe][:, :]], outs=[h1_tbl[pe][:, :]],
                    )

            # ---------------- phase 2: aggregate + KAN layer 2 -----------
            # software-pipelined: gathers/B loads issued LOOK windows ahead
            LOOK = 4
            inflight = {}

            def emit2(w):
                bt = bpool.tile([P, MAXSB, P], FP8, tag="b")
                nc.scalar.dma_start(
                    out=bt[:, :win_sb[w], :],
                    in_=Ball[:, win_sb0[w]:win_sb0[w] + win_sb[w], :])
                gts = []
                for (c, poff, n) in pieces_w[w]:
                    nsb = n // P
                    gt = gpool.tile([P, RMAX, HID_F], BF16, tag="g1")
                    nc.gpsimd.dma_gather(
                        gt[:, :nsb, :],
                        h1_tbl[c][:, :],
                        c_idx[:, poff // 16:(poff + n) // 16],
                        n, n, HID_F, queue_num=next_q())
                    gts.append((gt, poff, n))
                inflight[w] = (bt, gts)

            for w in range(min(LOOK, G)):
                emit2(w)
            for w in range(G):
                if w + LOOK < G:
                    emit2(w + LOOK)
                bt, gts = inflight.pop(w)
                pbin = ppool.tile([P, HID_F], F32, tag="big")
                nsb_tot = win_sb[w]
                si = 0
                for (gt, poff, n) in gts:
                    nsb = n // P
                    sb0 = poff // P - win_sb0[w]
                    for s in range(nsb):
                        nc.tensor.matmul(
                            out=pbin[:], lhsT=bt[:, sb0 + s, :],
                            rhs=gt[:, s, :],
                            start=(si == 0), stop=(si == nsb_tot - 1))
                        si += 1
                agg = wpool.tile([P, HID_F], F32, tag="agg")
                nc.vector.tensor_scalar_mul(agg[:], pbin[:], c_dr[:, w:w + 1])
                pt = ppool.tile([P, P], F32, tag="tr")
                nc.tensor.transpose(out=pt[:], in_=agg[:], identity=c_id[:])
                hT = wpool.tile([HID_F, P], BF16, tag="hT")
                nc.vector.tensor_scalar_mul(hT[:], pt[:], 1.0)
                q2 = wpool.tile([HID_F, P], BF16, tag="q2")
                nc.vector.tensor_tensor(out=q2[:], in0=hT[:], in1=hT[:],
                                        op=mybir.AluOpType.mult)
                q3 = wpool.tile([HID_F, P], BF16, tag="q3")
                nc.vector.tensor_tensor(out=q3[:], in0=q2[:], in1=hT[:],
                                        op=mybir.AluOpType.mult)
                ps2 = ppool.tile([P, OUT_F], F32, tag="small")
                nc.tensor.matmul(out=ps2[:], lhsT=hT[:], rhs=c_w2[0][:],
                                 start=True, stop=False)
                nc.tensor.matmul(out=ps2[:], lhsT=q2[:], rhs=c_w2[1][:],
                                 start=False, stop=False)
                nc.tensor.matmul(out=ps2[:], lhsT=q3[:], rhs=c_w2[2][:],
                                 start=False, stop=True)
                hb2 = wpool.tile([P, OUT_F], F32, tag="hb2")
                nc.vector.tensor_tensor(out=hb2[:], in0=ps2[:], in1=c_b2[:],
                                        op=mybir.AluOpType.add)
                pc = int(_PIECE_OF_W[w])
                r0 = (w - PIECE_W0[pc]) * P
                nc.scalar.dma_start(out=h2_in[pc][r0:r0 + P, :], in_=hb2[:])
                pe = piece_end_w(w)
                if pe is not None:
                    nc.gpsimd.collective_compute(
                        "AllGather", mybir.AluOpType.bypass,
                        replica_groups=[list(range(K))],
                        ins=[h2_in[pe][:, :]], outs=[h2_tbl[pe][:, :]],
                    )

            # ---------------- phase 3: aggregate + log_softmax -----------
            def emit3(w):
                bt = bpool.tile([P, MAXSB, P], FP8, tag="b")
                nc.scalar.dma_start(
                    out=bt[:, :win_sb[w], :],
                    in_=Ball[:, win_sb0[w]:win_sb0[w] + win_sb[w], :])
                gts = []
                for (c, poff, n) in pieces_w[w]:
                    nsb = n // P
                    gt = gpool.tile([P, RMAX, OUT_F], F32, tag="g2")
                    nc.gpsimd.dma_gather(
                        gt[:, :nsb, :],
                        h2_tbl[c][:, :],
                        c_idx[:, poff // 16:(poff + n) // 16],
                        n, n, OUT_F, queue_num=next_q())
                    gts.append((gt, poff, n))
                inflight[w] = (bt, gts)

            for w in range(min(LOOK, G)):
                emit3(w)
            for w in range(G):
                if w + LOOK < G:
                    emit3(w + LOOK)
                bt, gts = inflight.pop(w)
                pb3 = ppool.tile([P, OUT_F], F32, tag="small")
                nsb_tot = win_sb[w]
                si = 0
                for (gt, poff, n) in gts:
                    nsb = n // P
                    m16 = mpool.tile([P, RMAX, OUT_F], BF16, tag="m16")
                    nc.vector.tensor_scalar_mul(m16[:, :nsb, :],
                                                gt[:, :nsb, :], 1.0)
                    sb0 = poff // P - win_sb0[w]
                    for s in range(nsb):
                        nc.tensor.matmul(
                            out=pb3[:], lhsT=bt[:, sb0 + s, :],
                            rhs=m16[:, s, :],
                            start=(si == 0), stop=(si == nsb_tot - 1))
                        si += 1
                nc.vector.tensor_scalar_mul(tn_all[:, w, :], pb3[:],
                                            c_dr[:, w:w + 1])
                mx = wpool.tile([P, 1], F32, tag="mx")
                nc.vector.tensor_reduce(out=mx[:], in_=tn_all[:, w, :],
                                        axis=mybir.AxisListType.X,
                                        op=mybir.AluOpType.max)
                nc.vector.tensor_scalar_mul(nmx_all[:, w:w + 1], mx[:], -1.0)
                et = wpool.tile([P, OUT_F], F32, tag="et")
                nc.scalar.activation(out=et[:], in_=tn_all[:, w, :],
                                     func=mybir.ActivationFunctionType.Exp,
                                     bias=nmx_all[:, w:w + 1], scale=1.0,
                                     accum_out=se_all[:, w:w + 1])
            lse_all = cpool.tile([P, G], F32, tag="lse_all")
            nc.scalar.activation(out=lse_all[:], in_=se_all[:],
                                 func=mybir.ActivationFunctionType.Ln)
            for w in range(G):
                ot = wpool.tile([P, OUT_F], F32, tag="ot")
                nc.vector.tensor_scalar(ot[:], tn_all[:, w, :],
                                        nmx_all[:, w:w + 1],
                                        lse_all[:, w:w + 1],
                                        mybir.AluOpType.add,
                                        mybir.AluOpType.subtract)
                nc.sync.dma_start(out=y[w * P:(w + 1) * P, :], in_=ot[:])

    nc.compile()
    return nc


def kernel(x, edge_index, w1, b1, c1, w2, b2, c2):
    in_maps, meta = _host_prep(x, edge_index, w1, b1, c1, w2, b2, c2)
    nc = build_program(meta)
    res = run_bass_kernel_spmd(nc, in_maps, core_ids=list(range(K)))
    order = meta["order"]
    out = np.empty((N_NODES, OUT_F), dtype=np.float32)
    jr = np.arange(JREAL)
    for k in range(K):
        out[order[jr * K + k]] = res.results[k]["y"][:JREAL]
    return out


# revision 18
# speedup vs baseline: 1.5136x; 1.0629x over previous
"""KAN-GNN message passing on 8 TRN2 NeuronCores.

Strategy (data-parallel over nodes, per sharding hint):
 - Nodes ranked by in-degree, dealt round-robin to 8 cores (12544 local
   targets each, 98 windows of 128). The gathered tables are built as 4
   piece-wise AllGathers (windows 0-24, 25-49, 50-73, 74-97) so (a) each
   piece is <=25600 rows, addressable by the hardware dma_gather's int16
   indices, and (b) each collective overlaps with compute that produced
   or consumes the neighbouring pieces.
 - Phase 1: KAN layer 1 per local node (3 fused matmuls, bias+relu on
   DVE); AllGather piece p fires as soon as its windows are written.
 - Aggregation is edge-major: per core, in-edges sorted by (target
   window, source piece); each (w,c) run padded to a multiple of 128
   (shared across cores for SPMD) and fetched with hardware dma_gather
   (<=1024 rows per instruction, 4 SWDGE queues round-robin). A
   per-sub-batch 0/1 "binning" matrix B (fp8, streamed per window on the
   ACT engine's HWDGE) scatters each 128 gathered messages onto the
   window's 128 targets via one PE matmul accumulating in PSUM; the
   drain applies 1/deg on DVE.
 - KAN layer 2 per window (PE transpose, DVE powers, 3 matmuls); h2 kept
   f32 (256B rows) and AllGathered piece-wise during phase 2; second
   edge-major gather + binning pass; log_softmax with deferred Ln so the
   ACT engine loads each activation table once.
 - All indices/B matrices precomputed on host; per-core data as inputs.
"""
import numpy as np
import ml_dtypes

import concourse.bacc as bacc
import concourse.mybir as mybir
import concourse.tile as tile
import concourse.bass as bass
from concourse.bass_utils import run_bass_kernel_spmd

N_NODES = 100000
N_EDGES = 1600000
IN_F, HID_F, OUT_F = 128, 128, 64
K = 8               # cores
P = 128             # partitions
J = 12544           # local nodes per core (98*128), 12500 real + 44 pad
G = J // P          # 98 windows
JREAL = N_NODES // K
NCH = 4             # table pieces / gather chunks
PIECE_W = [25, 25, 24, 24]          # windows per piece
PIECE_W0 = [0, 25, 50, 74]          # first window of each piece
PIECE_ROWS = [w * P for w in PIECE_W]   # local rows per piece
RMAX = 8            # max sub-batches (of 128 edges) per gather piece

BF16 = mybir.dt.bfloat16
F32 = mybir.dt.float32
I16 = mybir.dt.int16
FP8 = mybir.dt.float8e4

_PIECE_OF_W = np.repeat(np.arange(NCH), PIECE_W)


def _host_prep(x, edge_index, w1, b1, c1, w2, b2, c2):
    src = np.asarray(edge_index[0], dtype=np.int64)
    tgt = np.asarray(edge_index[1], dtype=np.int64)
    x = np.asarray(x, dtype=np.float32)

    deg = np.bincount(tgt, minlength=N_NODES)
    order = np.argsort(-deg, kind="stable")
    rank_of = np.empty(N_NODES, dtype=np.int64)
    rank_of[order] = np.arange(N_NODES)
    core_of = rank_of % K
    j_of = rank_of // K

    # table position within its piece: rel = core*rows_p + (j - 128*w0_p)
    w_of = j_of // P
    piece_of = _PIECE_OF_W[w_of]
    rows_p = np.array(PIECE_ROWS)[piece_of]
    w0_p = np.array(PIECE_W0)[piece_of]
    rel_of = core_of * rows_p + (j_of - w0_p * P)

    # per-edge attributes
    ek = core_of[tgt]                    # owning core (by target)
    we = j_of[tgt] // P                  # target window
    ce = piece_of[src]                   # source chunk/piece
    rel = rel_of[src]                    # in-piece table row
    tcol = j_of[tgt] % P                 # target column within window

    key = (ek * G + we) * NCH + ce
    eorder = np.lexsort((rel, key))
    skey = key[eorder]
    counts = np.bincount(key, minlength=K * G * NCH).reshape(K, G, NCH)
    Lmax = counts.max(axis=0)                        # [G, NCH]
    Lpad = ((Lmax + P - 1) // P) * P                 # shared padded run lens

    run_off = np.zeros((G, NCH), dtype=np.int64)
    TOT = 0
    for w in range(G):
        for c in range(NCH):
            run_off[w, c] = TOT
            TOT += int(Lpad[w, c])
    SBTOT = TOT // P

    # place each core's sorted edges into the shared padded stream
    flat_counts = counts.reshape(-1)
    run_starts_e = np.concatenate([[0], np.cumsum(flat_counts)[:-1]])
    d_in_run = np.arange(len(skey)) - np.repeat(run_starts_e, flat_counts)
    kk = skey // (G * NCH)
    ww = (skey // NCH) % G
    cc = skey % NCH
    ppos = run_off[ww, cc] + d_in_run

    # pad entries point at consecutive rows 0,1,2,... so the DMA engine can
    # aggregate their descriptors into contiguous reads; B=0 kills the values
    idx_rel = np.tile((np.arange(TOT) % P).astype(np.int16), (K, 1))
    tcol_pad = np.full((K, TOT), -1, dtype=np.int64)
    idx_rel[kk, ppos] = rel[eorder].astype(np.int16)
    tcol_pad[kk, ppos] = tcol[eorder]

    # int16 index stream wrapped in 16 partitions, replicated x8 gpsimd cores
    blk = idx_rel.reshape(K, TOT // 16, 16).transpose(0, 2, 1)
    idx16 = np.ascontiguousarray(np.tile(blk, (1, 8, 1)))  # [K, 128, TOT//16]

    # binning matrices: Ball[k, p, s, t] = 1 if edge (s*128+p) targets t
    bm = np.zeros((K, TOT, P), dtype=ml_dtypes.float8_e4m3)
    kidx, eidx = np.nonzero(tcol_pad >= 0)
    bm[kidx, eidx, tcol_pad[kidx, eidx]] = 1.0
    Ball = np.ascontiguousarray(
        bm.reshape(K, SBTOT, P, P).transpose(0, 2, 1, 3))  # [K,128,SBTOT,128]

    # 1/deg per local target [K, P, G] (0 for pad targets)
    degs_kj = np.zeros((K, J), dtype=np.int64)
    degs_kj[core_of, j_of] = deg
    dr = 1.0 / np.maximum(degs_kj, 1).astype(np.float32)
    real = np.zeros((K, J), dtype=np.float32)
    real[:, :JREAL] = 1.0
    degrecip = np.ascontiguousarray(
        (dr * real).reshape(K, G, P).transpose(0, 2, 1))

    # xT shards, bf16 [K][IN_F, J]
    xT = np.zeros((K, IN_F, J), dtype=ml_dtypes.bfloat16)
    for k in range(K):
        nodes_k = order[np.arange(JREAL) * K + k]
        xT[k, :, :JREAL] = x[nodes_k].T.astype(ml_dtypes.bfloat16)

    # fused KAN weights
    A1 = (w1 + 0.1 * c1[:, :, 0]).astype(ml_dtypes.bfloat16)
    B1 = (0.1 * c1[:, :, 1]).astype(ml_dtypes.bfloat16)
    C1 = (0.1 * c1[:, :, 2]).astype(ml_dtypes.bfloat16)
    A2 = (w2 + 0.1 * c2[:, :, 0]).astype(ml_dtypes.bfloat16)
    B2 = (0.1 * c2[:, :, 1]).astype(ml_dtypes.bfloat16)
    C2 = (0.1 * c2[:, :, 2]).astype(ml_dtypes.bfloat16)
    b1b = np.tile(np.asarray(b1, np.float32)[None, :], (P, 1))
    b2b = np.tile(np.asarray(b2, np.float32)[None, :], (P, 1))
    ident = np.eye(P, dtype=np.float32)

    in_maps = []
    for k in range(K):
        in_maps.append({
            "xT": xT[k],
            "idx16": idx16[k],
            "Ball": Ball[k],
            "degrecip": degrecip[k],
            "A1": A1, "B1": B1, "C1": C1,
            "A2": A2, "B2": B2, "C2": C2,
            "b1b": b1b, "b2b": b2b, "ident": ident,
        })
    meta = {"Lpad": Lpad, "run_off": run_off, "TOT": TOT, "SBTOT": SBTOT,
            "order": order}
    return in_maps, meta


def build_program(meta):
    Lpad = meta["Lpad"]
    run_off = meta["run_off"]
    TOT = int(meta["TOT"])
    SBTOT = int(meta["SBTOT"])
    TOTC = TOT // 16

    nc = bacc.Bacc("TRN2", target_bir_lowering=False, debug=False, num_devices=K,
                   num_swdge_queues=4)

    xT = nc.dram_tensor("xT", [IN_F, J], BF16, kind="ExternalInput")
    idx16 = nc.dram_tensor("idx16", [P, TOTC], I16, kind="ExternalInput")
    Ball = nc.dram_tensor("Ball", [P, SBTOT, P], FP8, kind="ExternalInput")
    degrecip = nc.dram_tensor("degrecip", [P, G], F32, kind="ExternalInput")
    A1 = nc.dram_tensor("A1", [IN_F, HID_F], BF16, kind="ExternalInput")
    B1 = nc.dram_tensor("B1", [IN_F, HID_F], BF16, kind="ExternalInput")
    C1 = nc.dram_tensor("C1", [IN_F, HID_F], BF16, kind="ExternalInput")
    A2 = nc.dram_tensor("A2", [HID_F, OUT_F], BF16, kind="ExternalInput")
    B2 = nc.dram_tensor("B2", [HID_F, OUT_F], BF16, kind="ExternalInput")
    C2 = nc.dram_tensor("C2", [HID_F, OUT_F], BF16, kind="ExternalInput")
    b1b = nc.dram_tensor("b1b", [P, HID_F], F32, kind="ExternalInput")
    b2b = nc.dram_tensor("b2b", [P, OUT_F], F32, kind="ExternalInput")
    ident = nc.dram_tensor("ident", [P, P], F32, kind="ExternalInput")
    y = nc.dram_tensor("y", [J, OUT_F], F32, kind="ExternalOutput")

    h1_in = []
    h1_tbl = []
    h2_in = []
    h2_tbl = []
    for p in range(NCH):
        rp = PIECE_ROWS[p]
        h1_in.append(nc.dram_tensor(f"h1_in{p}", [rp, HID_F], BF16,
                                    kind="Internal"))
        h1_tbl.append(nc.dram_tensor(f"h1_tbl{p}", [K * rp, HID_F], BF16,
                                     kind="Internal", addr_space="Shared"))
        h2_in.append(nc.dram_tensor(f"h2_in{p}", [rp, OUT_F], F32,
                                    kind="Internal"))
        h2_tbl.append(nc.dram_tensor(f"h2_tbl{p}", [K * rp, OUT_F], F32,
                                     kind="Internal", addr_space="Shared"))

    # pieces per window: (chunk, padded-stream offset, n)
    pieces_w = []
    win_sb0 = []
    win_sb = []
    for w in range(G):
        pieces = []
        for c in range(NCH):
            L = int(Lpad[w, c])
            off = int(run_off[w, c])
            o = 0
            while o < L:
                n = min(L - o, RMAX * P)
                pieces.append((c, off + o, n))
                o += n
        pieces_w.append(pieces)
        win_sb0.append(int(run_off[w, 0]) // P)
        win_sb.append(sum(int(Lpad[w, c]) for c in range(NCH)) // P)
    MAXSB = max(win_sb)

    qctr = [0]

    def next_q():
        q = qctr[0] % 4
        qctr[0] += 1
        return q

    def piece_end_w(w):
        for p in range(NCH):
            if w == PIECE_W0[p] + PIECE_W[p] - 1:
                return p
        return None

    with tile.TileContext(nc) as tc:
        with (
            tc.tile_pool(name="consts", bufs=1) as cpool,
            tc.tile_pool(name="work", bufs=3) as wpool,
            tc.tile_pool(name="gath", bufs=16) as gpool,
            tc.tile_pool(name="m16p", bufs=6) as mpool,
            tc.tile_pool(name="bmat", bufs=5) as bpool,
            tc.tile_pool(name="psum", bufs=2, space="PSUM") as ppool,
        ):
            c_idx = cpool.tile([P, TOTC], I16, tag="idx16")
            nc.sync.dma_start(out=c_idx[:], in_=idx16[:, :])
            c_dr = cpool.tile([P, G], F32, tag="dr")
            nc.sync.dma_start(out=c_dr[:], in_=degrecip[:, :])
            c_w1 = []
            for nm, t in (("A1", A1), ("B1", B1), ("C1", C1)):
                wt = cpool.tile([IN_F, HID_F], BF16, tag=nm)
                nc.sync.dma_start(out=wt[:], in_=t[:, :])
                c_w1.append(wt)
            c_w2 = []
            for nm, t in (("A2", A2), ("B2", B2), ("C2", C2)):
                wt = cpool.tile([HID_F, OUT_F], BF16, tag=nm)
                nc.sync.dma_start(out=wt[:], in_=t[:, :])
                c_w2.append(wt)
            c_b1 = cpool.tile([P, HID_F], F32, tag="b1b")
            nc.sync.dma_start(out=c_b1[:], in_=b1b[:, :])
            c_b2 = cpool.tile([P, OUT_F], F32, tag="b2b")
            nc.sync.dma_start(out=c_b2[:], in_=b2b[:, :])
            c_id = cpool.tile([P, P], F32, tag="ident")
            nc.sync.dma_start(out=c_id[:], in_=ident[:, :])
            # per-window softmax state, filled in phase 3
            tn_all = cpool.tile([P, G, OUT_F], F32, tag="tn_all")
            nmx_all = cpool.tile([P, G], F32, tag="nmx_all")
            se_all = cpool.tile([P, G], F32, tag="se_all")

            # ---------------- phase 1: KAN layer 1 on the shard ----------
            for w in range(G):
                pc = int(_PIECE_OF_W[w])
                r0 = (w - PIECE_W0[pc]) * P
                xt = wpool.tile([IN_F, P], BF16, tag="xt")
                nc.sync.dma_start(out=xt[:], in_=xT[:, w * P:(w + 1) * P])
                x2 = wpool.tile([IN_F, P], BF16, tag="x2")
                nc.vector.tensor_tensor(out=x2[:], in0=xt[:], in1=xt[:],
                                        op=mybir.AluOpType.mult)
                x3 = wpool.tile([IN_F, P], BF16, tag="x3")
                nc.vector.tensor_tensor(out=x3[:], in0=x2[:], in1=xt[:],
                                        op=mybir.AluOpType.mult)
                ps = ppool.tile([P, HID_F], F32, tag="big")
                nc.tensor.matmul(out=ps[:], lhsT=xt[:], rhs=c_w1[0][:],
                                 start=True, stop=False)
                nc.tensor.matmul(out=ps[:], lhsT=x2[:], rhs=c_w1[1][:],
                                 start=False, stop=False)
                nc.tensor.matmul(out=ps[:], lhsT=x3[:], rhs=c_w1[2][:],
                                 start=False, stop=True)
                hb = wpool.tile([P, HID_F], F32, tag="hb")
                nc.vector.tensor_tensor(out=hb[:], in0=ps[:], in1=c_b1[:],
                                        op=mybir.AluOpType.add)
                h1t = wpool.tile([P, HID_F], BF16, tag="h1t")
                nc.vector.tensor_scalar_max(h1t[:], hb[:], 0.0)
                nc.sync.dma_start(out=h1_in[pc][r0:r0 + P, :], in_=h1t[:])
                pe = piece_end_w(w)
                if pe is not None:
                    nc.gpsimd.collective_compute(
                        "AllGather", mybir.AluOpType.bypass,
                        replica_groups=[list(range(K))],
                        ins=[h1_in[pe][:, :]], outs=[h1_tbl[pe][:, :]],
                    )

            # ---------------- phase 2: aggregate + KAN layer 2 -----------
            # software-pipelined: gathers/B loads issued LOOK windows ahead
            LOOK = 4
            inflight = {}

            def emit2(w):
                bt = bpool.tile([P, MAXSB, P], FP8, tag="b")
                nc.scalar.dma_start(
                    out=bt[:, :win_sb[w], :],
                    in_=Ball[:, win_sb0[w]:win_sb0[w] + win_sb[w], :])
                gts = []
                for (c, poff, n) in pieces_w[w]:
                    nsb = n // P
                    gt = gpool.tile([P, RMAX, HID_F], BF16, tag="g1")
                    nc.gpsimd.dma_gather(
                        gt[:, :nsb, :],
                        h1_tbl[c][:, :],
                        c_idx[:, poff // 16:(poff + n) // 16],
                        n, n, HID_F, queue_num=next_q())
                    gts.append((gt, poff, n))
                inflight[w] = (bt, gts)

            for w in range(min(LOOK, G)):
                emit2(w)
            for w in range(G):
                if w + LOOK < G:
                    emit2(w + LOOK)
                bt, gts = inflight.pop(w)
                pbin = ppool.tile([P, HID_F], F32, tag="big")
                nsb_tot = win_sb[w]
                si = 0
                for (gt, poff, n) in gts:
                    nsb = n // P
                    sb0 = poff // P - win_sb0[w]
                    for s in range(nsb):
                        nc.tensor.matmul(
                            out=pbin[:], lhsT=bt[:, sb0 + s, :],
                            rhs=gt[:, s, :],
                            start=(si == 0), stop=(si == nsb_tot - 1))
                        si += 1
                agg = wpool.tile([P, HID_F], F32, tag="agg")
                nc.vector.tensor_scalar_mul(agg[:], pbin[:], c_dr[:, w:w + 1])
                pt = ppool.tile([P, P], F32, tag="tr")
                nc.tensor.transpose(out=pt[:], in_=agg[:], identity=c_id[:])
                hT = wpool.tile([HID_F, P], BF16, tag="hT")
                nc.vector.tensor_scalar_mul(hT[:], pt[:], 1.0)
                q2 = wpool.tile([HID_F, P], BF16, tag="q2")
                nc.vector.tensor_tensor(out=q2[:], in0=hT[:], in1=hT[:],
                                        op=mybir.AluOpType.mult)
                q3 = wpool.tile([HID_F, P], BF16, tag="q3")
                nc.vector.tensor_tensor(out=q3[:], in0=q2[:], in1=hT[:],
                                        op=mybir.AluOpType.mult)
                ps2 = ppool.tile([P, OUT_F], F32, tag="small")
                nc.tensor.matmul(out=ps2[:], lhsT=hT[:], rhs=c_w2[0][:],
                                 start=True, stop=False)
                nc.tensor.matmul(out=ps2[:], lhsT=q2[:], rhs=c_w2[1][:],
                                 start=False, stop=False)
                nc.tensor.matmul(out=ps2[:], lhsT=q3[:], rhs=c_w2[2][:],
                                 start=False, stop=True)
                hb2 = wpool.tile([P, OUT_F], F32, tag="hb2")
                nc.vector.tensor_tensor(out=hb2[:], in0=ps2[:], in1=c_b2[:],
                                        op=mybir.AluOpType.add)
                pc = int(_PIECE_OF_W[w])
                r0 = (w - PIECE_W0[pc]) * P
                nc.scalar.dma_start(out=h2_in[pc][r0:r0 + P, :], in_=hb2[:])
                pe = piece_end_w(w)
                if pe is not None:
                    nc.gpsimd.collective_compute(
                        "AllGather", mybir.AluOpType.bypass,
                        replica_groups=[list(range(K))],
                        ins=[h2_in[pe][:, :]], outs=[h2_tbl[pe][:, :]],
                    )

            # ---------------- phase 3: aggregate + log_softmax -----------
            def emit3(w):
                bt = bpool.tile([P, MAXSB, P], FP8, tag="b")
                nc.scalar.dma_start(
                    out=bt[:, :win_sb[w], :],
                    in_=Ball[:, win_sb0[w]:win_sb0[w] + win_sb[w], :])
                gts = []
                for (c, poff, n) in pieces_w[w]:
                    nsb = n // P
                    gt = gpool.tile([P, RMAX, OUT_F], F32, tag="g2")
                    nc.gpsimd.dma_gather(
                        gt[:, :nsb, :],
                        h2_tbl[c][:, :],
                        c_idx[:, poff // 16:(poff + n) // 16],
                        n, n, OUT_F, queue_num=next_q())
                    gts.append((gt, poff, n))
                inflight[w] = (bt, gts)

            for w in range(min(LOOK, G)):
                emit3(w)
            for w in range(G):
                if w + LOOK < G:
                    emit3(w + LOOK)
                bt, gts = inflight.pop(w)
                pb3 = ppool.tile([P, OUT_F], F32, tag="small")
                nsb_tot = win_sb[w]
                si = 0
                for (gt, poff, n) in gts:
                    nsb = n // P
                    m16 = mpool.tile([P, RMAX, OUT_F], BF16, tag="m16")
                    nc.vector.tensor_scalar_mul(m16[:, :nsb, :],
                                                gt[:, :nsb, :], 1.0)
                    sb0 = poff // P - win_sb0[w]
                    for s in range(nsb):
                        nc.tensor.matmul(
                            out=pb3[:], lhsT=bt[:, sb0 + s, :],
                            rhs=m16[:, s, :],
                            start=(si == 0), stop=(si == nsb_tot - 1))
                        si += 1
                nc.vector.tensor_scalar_mul(tn_all[:, w, :], pb3[:],
                                            c_dr[:, w:w + 1])
                mx = wpool.tile([P, 1], F32, tag="mx")
                nc.vector.tensor_reduce(out=mx[:], in_=tn_all[:, w, :],
                                        axis=mybir.AxisListType.X,
                                        op=mybir.AluOpType.max)
                nc.vector.tensor_scalar_mul(nmx_all[:, w:w + 1], mx[:], -1.0)
                et = wpool.tile([P, OUT_F], F32, tag="et")
                nc.scalar.activation(out=et[:], in_=tn_all[:, w, :],
                                     func=mybir.ActivationFunctionType.Exp,
                                     bias=nmx_all[:, w:w + 1], scale=1.0,
                                     accum_out=se_all[:, w:w + 1])
            lse_all = cpool.tile([P, G], F32, tag="lse_all")
            nc.scalar.activation(out=lse_all[:], in_=se_all[:],
                                 func=mybir.ActivationFunctionType.Ln)
            for w in range(G):
                ot = wpool.tile([P, OUT_F], F32, tag="ot")
                nc.vector.tensor_scalar(ot[:], tn_all[:, w, :],
                                        nmx_all[:, w:w + 1],
                                        lse_all[:, w:w + 1],
                                        mybir.AluOpType.add,
                                        mybir.AluOpType.subtract)
                nc.sync.dma_start(out=y[w * P:(w + 1) * P, :], in_=ot[:])

    nc.compile()
    return nc


def kernel(x, edge_index, w1, b1, c1, w2, b2, c2):
    in_maps, meta = _host_prep(x, edge_index, w1, b1, c1, w2, b2, c2)
    nc = build_program(meta)
    res = run_bass_kernel_spmd(nc, in_maps, core_ids=list(range(K)))
    order = meta["order"]
    out = np.empty((N_NODES, OUT_F), dtype=np.float32)
    jr = np.arange(JREAL)
    for k in range(K):
        out[order[jr * K + k]] = res.results[k]["y"][:JREAL]
    return out
